# revision 2
# baseline (speedup 1.0000x reference)
"""Causal multi-head attention for Trainium2 (Bass/Tile), 8 NeuronCores. v4.

Problem: q,k,v [B=4, H=16, S=2048, d=64] fp32;
         out = softmax(causal_mask(QK^T/sqrt(d))) @ V.
Sharding: 64 (b,h) head-slices, 8 per core (head parallelism, no comms).

v4 changes over v3 (from trace analysis of the 251.8us baseline):
  - The PE idled ~47us in >1us gaps (startup + head-pair boundaries), which
    also re-armed the HAM throttle (99us at K=4/8, half clock).  Fixes:
    * startup: strip XBARs are issued before const building, sliced fine
      (512 cols) with k-before-q so the first QK can start at ~2-3us;
      warm-up matmul count cut accordingly.
    * compacts moved off the sync queue (to gpsimd swdge) so XBARs,
      compacts and output stores don't serialize on one queue.
  - v is now loaded with a CASTING swdge DMA (fp32 HBM -> bf16 SBUF)
    straight into the PV weight layout: the vsb staging tile and the
    ~18us of DVE cast work are gone.
  - exp always emits ONE ACT/DVE instruction per (m,j) step (the j0>0
    diagonal case used two): the rectangle [128, 2, QB-j0] includes a
    dead 128-col head on plane 1 (masked region, never read by PV) but
    saves the 352-cycle ACT fixed overhead.
  - both-diagonal tri-masks are fused into ONE gpsimd op using a
    [128, 2, 256] const (plane0 = tri|ones, plane1 = ones|tri).
  - the 4 per-block output scales (otr * 1/sum) are fused into one DVE
    tensor_tensor with a broadcast AP (was 4 ops x 283ns).
"""

import math
import os

import numpy as np

import concourse.bacc as bacc
import concourse.bass as bass
import concourse.mybir as mybir
from concourse.bass_utils import run_bass_kernel_spmd
from concourse.masks import make_identity, make_upper_triangular
from concourse.tile import TileContext

B, H, S, D = 4, 16, 2048, 64
NCORES = 8
HPC = (B * H) // NCORES  # 8 heads per core
QB = 512                 # q-block (one PSUM bank of fp32)
KC = 128                 # k-chunk
NQB = S // QB            # 4
NT = S // 128            # 16

FP32 = mybir.dt.float32
BF16 = mybir.dt.bfloat16
I16 = mybir.dt.int16

# schraudolph: bf16_bits(exp(x*0.125)) ~= x * SCH_A + SCH_B
SCH_A = 128.0 / math.log(2.0) * 0.125
SCH_B = 16248.5
SCH_FRAC = float(os.environ.get("SCH_FRAC", "0.34"))
WARM_MM = int(os.environ.get("WARM_MM", "14"))
MASK_POOL = bool(int(os.environ.get("MASK_POOL", "1")))
PEND_DEPTH = int(os.environ.get("PEND_DEPTH", "4"))


def build_program() -> bass.Bass:
    nc = bacc.Bacc(None, target_bir_lowering=False, debug=False)

    q_in = nc.declare_dram_parameter("q", [HPC, S, D], FP32, isOutput=False)
    k_in = nc.declare_dram_parameter("k", [HPC, S, D], FP32, isOutput=False)
    v_in = nc.declare_dram_parameter("v", [HPC, S, D], FP32, isOutput=False)
    out_p = nc.declare_dram_parameter("out", [HPC, S, D], FP32, isOutput=True)

    with TileContext(nc) as tc:
        with (
            tc.tile_pool(name="consts", bufs=1) as consts,
            tc.tile_pool(name="inp", bufs=2) as inp,
            tc.tile_pool(name="strip", bufs=2) as strip,
            tc.tile_pool(name="ppool", bufs=6) as ppool,
            tc.tile_pool(name="osb", bufs=3) as osb,
            tc.tile_pool(name="res", bufs=4) as res,
            tc.tile_pool(name="ps_s", bufs=3, space="PSUM") as ps_s,
            tc.tile_pool(name="ps_o", bufs=2, space="PSUM") as ps_o,
        ):
            # ---------------- prep helpers -------------------------------
            def prep_strip_slice(st, raws, name, src, j, sl):
                """XBAR-transpose one column slice of head j's q/k strip.

                fp32 [S, 64] bitcast to bf16 [S, 128]: halfword column
                c = 2d+h of row s, h=1 is the truncated-bf16 plane.  The
                XBAR lands column c on partition c; a swdge compact then
                moves the odd partitions into the shared strip tile
                (j=0 -> partitions 0:64, j=1 -> 64:128).
                """
                raw = raws[(name, j)]
                nc.sync.dma_start_transpose(
                    raw[:, sl], src[j].bitcast(BF16)[sl, :]
                )
                nc.gpsimd.dma_start(
                    out=st[name][64 * j : 64 * (j + 1), sl],
                    in_=raw.rearrange("(d h) s -> h d s", h=2)[1][:, sl],
                )

            def prep_strips(i, st, j):
                h = 2 * i + j
                for name, src in (("qT", q_in), ("kT", k_in)):
                    raw = strip.tile([128, S], BF16, tag=f"raw{name}{j}",
                                     name=f"raw{name}{j}")
                    t = st.get(name)
                    if t is None:
                        t = strip.tile([128, S], BF16, tag=name, name=name)
                        st[name] = t
                    nc.sync.dma_start_transpose(raw, src[h].bitcast(BF16))
                    nc.gpsimd.dma_start(
                        out=t[64 * j : 64 * (j + 1), :],
                        in_=raw.rearrange("(d h) s -> h d s", h=2)[1],
                    )

            def prep_loads_v(i, st, ones_c):
                # casting swdge DMA: fp32 HBM -> bf16 SBUF, PV weight layout
                for j, h in enumerate((2 * i, 2 * i + 1)):
                    vb = inp.tile(
                        [128, NT, D + 1], BF16, tag="vb", name=f"vb{j}", bufs=4
                    )
                    nc.vector.tensor_copy(vb[:, :, D], ones_c)
                    nc.gpsimd.dma_start(
                        out=vb[:, :, 0:D],
                        in_=v_in[h].rearrange("(t p) d -> p t d", p=128),
                    )
                    st[f"vb{j}"] = vb

            # ---------------- pair-0 prep + consts, interleaved ----------
            st_cur = {}
            raws0 = {}
            for name in ("qT", "kT"):
                st_cur[name] = strip.tile([128, S], BF16, tag=name, name=name)
                for j in range(2):
                    raws0[(name, j)] = strip.tile(
                        [128, S], BF16, tag=f"raw{name}{j}",
                        name=f"raw{name}{j}",
                    )
            # first 512 columns of k then q, both heads: unblocks QK b=0
            for name, src in (("kT", k_in), ("qT", q_in)):
                for j in range(2):
                    prep_strip_slice(st_cur, raws0, name, src, j, slice(0, 512))

            # consts (gpsimd/DVE) while the first XBARs fly
            tri_f32 = consts.tile([128, 128], FP32)
            make_upper_triangular(nc, tri_f32, val=1.0, diag=True)
            tri = consts.tile([128, 128], BF16)
            nc.vector.tensor_copy(tri, tri_f32)
            # two-plane diagonal mask: plane0 = tri|ones, plane1 = ones|tri
            tri2 = consts.tile([128, 2, 256], BF16)
            nc.vector.memset(tri2, 1.0)
            nc.vector.tensor_copy(tri2[:, 0, 0:128], tri)
            nc.vector.tensor_copy(tri2[:, 1, 128:256], tri)
            ones_c = consts.tile([128, NT], FP32)
            nc.vector.memset(ones_c, 1.0)
            warm = consts.tile([1, 8], FP32)
            nc.scalar.activation(
                warm, ones_c[0:1, 0:8], mybir.ActivationFunctionType.Exp
            )

            prep_loads_v(0, st_cur, ones_c)

            ident = consts.tile([128, 128], FP32)
            make_identity(nc, ident)
            identb = consts.tile([128, 128], BF16)
            nc.vector.tensor_copy(identb, ident)

            # PE clock warm-up bridging until the first strips land
            wtp = ps_s.tile([128, 2048], BF16, tag="sP", name="wtp")
            for i in range(WARM_MM):
                off = 512 * (i % 4)
                nc.tensor.transpose(wtp[:, off : off + 128], identb, identb)

            # rest of the pair-0 strips
            for sl in (slice(512, 1024), slice(1024, 2048)):
                for name, src in (("kT", k_in), ("qT", q_in)):
                    for j in range(2):
                        prep_strip_slice(st_cur, raws0, name, src, j, sl)

            # ---------------- main loop pieces ---------------------------
            sch_state = [0.0, 0.0]  # [total_cols, dve_cols]

            def pick_producer(cols):
                sch_state[0] += cols
                if sch_state[1] < SCH_FRAC * sch_state[0]:
                    sch_state[1] += cols
                    return "dve"
                return "act"

            def emit_qk(st, j, b, m):
                qT, kT = st["qT"], st["kT"]
                o = 64 * j
                cs = (2 * m, 2 * m + 1)
                ts = [c - 4 * b for c in cs]
                j0s = [128 * t if t >= 0 else 0 for t in ts]
                sP = ps_s.tile([128, 2, QB], FP32, tag="sP", name="sP")
                for x in range(2):
                    nc.tensor.matmul(
                        sP[:, x, j0s[x] : QB],
                        kT[o : o + 64, KC * cs[x] : KC * (cs[x] + 1)],
                        qT[o : o + 64, QB * b + j0s[x] : QB * (b + 1)],
                        start=True,
                        stop=True,
                    )
                return sP, j0s, ts

            def emit_exp(sP, j0s, ts):
                pTi = ppool.tile([128, 2, QB], I16, tag="pT", name="pT")
                pT = pTi.bitcast(BF16)

                def emit_one(dst_bf, dst_i16, src, cols):
                    # GPSIMD cannot read PSUM, so producers are ACT/DVE only
                    prod = pick_producer(cols)
                    if prod == "act":
                        nc.scalar.activation(
                            dst_bf, src,
                            mybir.ActivationFunctionType.Exp, scale=0.125,
                        )
                    else:
                        nc.vector.tensor_scalar(
                            dst_i16, src, SCH_A, SCH_B,
                            mybir.AluOpType.mult, mybir.AluOpType.add,
                        )

                if j0s[0] == 0:
                    # full rectangle (plane-1 head may be dead; PV skips it)
                    emit_one(
                        pT.rearrange("p a f -> p (a f)"),
                        pTi.rearrange("p a f -> p (a f)"),
                        sP.rearrange("p a f -> p (a f)"),
                        2 * QB,
                    )
                else:
                    # both planes from j0s[0]; plane-1 cols j0:j0+128 are
                    # dead (strictly-masked region, never read by PV)
                    j0 = j0s[0]
                    emit_one(
                        pT[:, :, j0:QB], pTi[:, :, j0:QB], sP[:, :, j0:QB],
                        2 * (QB - j0),
                    )
                eng = nc.gpsimd if MASK_POOL else nc.vector
                if ts[0] >= 0:
                    # both diagonal: one fused op over [128, 2, 256]
                    j0 = j0s[0]
                    eng.tensor_mul(
                        pT[:, :, j0 : j0 + 256],
                        pT[:, :, j0 : j0 + 256],
                        tri2,
                    )
                elif ts[1] >= 0:
                    eng.tensor_mul(
                        pT[:, 1, j0s[1] : j0s[1] + 128],
                        pT[:, 1, j0s[1] : j0s[1] + 128],
                        tri,
                    )
                return pT

            def emit_pv(vb, oT, pT, j0s, m, npairs):
                cs = (2 * m, 2 * m + 1)
                for x in range(2):
                    nc.tensor.matmul(
                        oT[:, j0s[x] : QB],
                        vb[:, cs[x]],
                        pT[:, x, j0s[x] : QB],
                        start=(m == 0 and x == 0),
                        stop=(m == npairs - 1 and x == 1),
                    )

            def emit_output(h, b, oT):
                oTc = osb.tile([D + 1, QB], BF16, name="oTc")
                nc.vector.tensor_copy(oTc, oT)
                otr = ps_s.tile(
                    [128, 4, D + 1], BF16, tag="sP", name="otr",
                    padded_shape=[128, 4, 512],
                )
                for i in range(4):
                    nc.tensor.transpose(
                        otr[:, i],
                        oTc[:, 128 * i : 128 * (i + 1)],
                        identb[0 : D + 1, 0 : D + 1],
                    )
                rec = res.tile([128, 4], FP32, name="rec")
                nc.vector.reciprocal(rec, otr[:, :, D])
                ores = res.tile([128, 4, D], FP32, name="ores")
                nc.vector.tensor_mul(
                    ores,
                    otr[:, :, 0:D],
                    rec.unsqueeze(2).broadcast_to([128, 4, D]),
                )
                nc.sync.dma_start(
                    out=out_p[h, QB * b : QB * (b + 1), :].rearrange(
                        "(t p) d -> p t d", p=128
                    ),
                    in_=ores,
                )

            # ---------------- schedule -----------------------------------
            NP = HPC // 2
            deferred_prev = []
            pend = []  # (vb, oT, pT, j0s, m, npairs): PV deferred one slot
            for i in range(NP):
                st_nxt = {} if i + 1 < NP else None
                deferred = []
                for b in range(NQB):
                    npairs = 2 * (b + 1)
                    oTs = [
                        ps_o.tile([D + 1, QB], FP32, tag="oT", name=f"oT{j}")
                        for j in range(2)
                    ]
                    gb = i * NQB + b
                    order = [(m, j) for m in range(npairs) for j in range(2)]
                    for m, j in order:
                        sP, j0s, ts = emit_qk(st_cur, j, b, m)
                        pT = emit_exp(sP, j0s, ts)
                        pend.append(
                            (st_cur[f"vb{j}"], oTs[j], pT, j0s, m, npairs, gb)
                        )
                        if len(pend) > PEND_DEPTH:
                            emit_pv(*pend.pop(0)[:6])
                        if m <= 1 and j == 1 and (deferred_prev or deferred):
                            # stagger the two heads' output stages (m=0 and
                            # m=1) so only one otr occupies an sP slot at a
                            # time; drain the previous block's deferred PVs
                            # first or the in-order PE queue deadlocks
                            while pend and pend[0][6] < gb:
                                emit_pv(*pend.pop(0)[:6])
                            todo = deferred_prev + deferred
                            deferred_prev = []
                            deferred = []
                            emit_output(*todo[0])
                            if m == 1 or npairs == 2:
                                for args in todo[1:]:
                                    emit_output(*args)
                            else:
                                deferred = todo[1:]
                    # prep interleave points at block boundaries
                    if st_nxt is not None:
                        if b == 0:
                            prep_strips(i + 1, st_nxt, 0)
                        elif b == 1:
                            prep_loads_v(i + 1, st_nxt, ones_c)
                            prep_strips(i + 1, st_nxt, 1)
                    deferred = [(2 * i + j, b, oTs[j]) for j in range(2)]
                deferred_prev = deferred
                st_cur = st_nxt
            while pend:
                emit_pv(*pend.pop(0)[:6])
            for args in deferred_prev:
                emit_output(*args)

    nc.compile()
    return nc


_NC_CACHE = None
LAST_RESULT = None


def kernel(q: np.ndarray, k: np.ndarray, v: np.ndarray) -> np.ndarray:
    global _NC_CACHE, LAST_RESULT
    if _NC_CACHE is None:
        _NC_CACHE = build_program()
    nc = _NC_CACHE

    def shard(x):
        x = np.ascontiguousarray(np.asarray(x, dtype=np.float32)).reshape(
            B * H, S, D
        )
        return [
            np.ascontiguousarray(x[i * HPC : (i + 1) * HPC])
            for i in range(NCORES)
        ]

    qs, ks, vs = shard(q), shard(k), shard(v)
    ncores = int(os.environ.get("KCORES", str(NCORES)))
    in_maps = [{"q": qs[i], "k": ks[i], "v": vs[i]} for i in range(NCORES)]
    trace = bool(int(os.environ.get("KERNEL_TRACE", "0")))
    result = run_bass_kernel_spmd(
        nc, in_maps[:ncores], core_ids=list(range(ncores)), trace=trace
    )
    LAST_RESULT = result
    outs = [r["out"] for r in result.results]
    if ncores < NCORES:
        outs += [np.zeros((HPC, S, D), np.float32)] * (NCORES - ncores)
    out = np.concatenate(outs, axis=0)
    return out.reshape(B, H, S, D)


# revision 3
# speedup vs baseline: 1.3274x; 1.3274x over previous
"""Causal multi-head attention for Trainium2 (Bass/Tile), 8 NeuronCores. v5.

Problem: q,k,v [B=4, H=16, S=2048, d=64] fp32;
         out = softmax(causal_mask(QK^T/sqrt(d))) @ V.
Sharding: 64 (b,h) head-slices, 8 per core (head parallelism, no comms).

v5 = v3 + the changes that measured well in v4, minus the ones that didn't:
  - startup: the pair-0 strip XBARs+compacts are split between the two
    HWDGE queues (k on sync, q on scalar - idle until the first exp) with
    a fine 512-col first slice, and issued before const building, so the
    first QK starts at ~4us instead of ~30us.  Warm-up matmuls trimmed.
  - v is loaded with a CASTING swdge DMA (fp32 HBM -> bf16 SBUF) straight
    into the PV weight layout (saves ~18us of DVE cast work).
  - the 4 per-block output scales (otr * 1/sum) are one DVE tensor_tensor
    with a broadcast AP (was 4 ops; saved ~20us DVE).
  - reverted from v4: fused tri2 masks (gpsimd is ~2.1ns/col - doubling
    the masked area lost), 3D-strided exp merge (ACT/DVE slow down on
    strided PSUM reads), swdge compacts (2.7x slower than hwdge).
"""

import math
import os

import numpy as np

import concourse.bacc as bacc
import concourse.bass as bass
import concourse.mybir as mybir
from concourse.bass_utils import run_bass_kernel_spmd
from concourse.masks import make_identity, make_upper_triangular
from concourse.tile import TileContext

B, H, S, D = 4, 16, 2048, 64
NCORES = 8
HPC = (B * H) // NCORES  # 8 heads per core
QB = 512                 # q-block (one PSUM bank of fp32)
KC = 128                 # k-chunk
NQB = S // QB            # 4
NT = S // 128            # 16

FP32 = mybir.dt.float32
BF16 = mybir.dt.bfloat16
I16 = mybir.dt.int16

# schraudolph: bf16_bits(exp(x*0.125)) ~= x * SCH_A + SCH_B
SCH_A = 128.0 / math.log(2.0) * 0.125
SCH_B = 16248.5
SCH_FRAC = float(os.environ.get("SCH_FRAC", "0.34"))
WARM_MM = int(os.environ.get("WARM_MM", "10"))
MASK_POOL = bool(int(os.environ.get("MASK_POOL", "1")))
PEND_DEPTH = int(os.environ.get("PEND_DEPTH", "4"))


def build_program() -> bass.Bass:
    nc = bacc.Bacc(None, target_bir_lowering=False, debug=False)

    q_in = nc.declare_dram_parameter("q", [HPC, S, D], FP32, isOutput=False)
    k_in = nc.declare_dram_parameter("k", [HPC, S, D], FP32, isOutput=False)
    v_in = nc.declare_dram_parameter("v", [HPC, S, D], FP32, isOutput=False)
    out_p = nc.declare_dram_parameter("out", [HPC, S, D], FP32, isOutput=True)

    with TileContext(nc) as tc:
        with (
            tc.tile_pool(name="consts", bufs=1) as consts,
            tc.tile_pool(name="inp", bufs=2) as inp,
            tc.tile_pool(name="strip", bufs=2) as strip,
            tc.tile_pool(name="ppool", bufs=6) as ppool,
            tc.tile_pool(name="osb", bufs=3) as osb,
            tc.tile_pool(name="res", bufs=4) as res,
            tc.tile_pool(name="ps_s", bufs=3, space="PSUM") as ps_s,
            tc.tile_pool(name="ps_o", bufs=2, space="PSUM") as ps_o,
        ):
            # ---------------- prep helpers -------------------------------
            def prep_strip_slice(eng, st, raws, name, src, j, sl):
                """XBAR-transpose one column slice of head j's q/k strip on
                the given HWDGE engine queue, then compact on the same
                queue (hwdge: ~0.65us/full-strip vs 1.7us on swdge).

                fp32 [S, 64] bitcast to bf16 [S, 128]: halfword column
                c = 2d+h of row s, h=1 is the truncated-bf16 plane.  The
                XBAR lands column c on partition c; the compact moves the
                odd partitions into the shared strip tile (j=0 ->
                partitions 0:64, j=1 -> 64:128).
                """
                raw = raws[(name, j)]
                eng.dma_start_transpose(raw[:, sl], src[j].bitcast(BF16)[sl, :])
                eng.dma_start(
                    out=st[name][64 * j : 64 * (j + 1), sl],
                    in_=raw.rearrange("(d h) s -> h d s", h=2)[1][:, sl],
                )

            def prep_strips(i, st, j):
                h = 2 * i + j
                for name, src in (("kT", k_in), ("qT", q_in)):
                    raw = strip.tile([128, S], BF16, tag=f"raw{name}{j}",
                                     name=f"raw{name}{j}")
                    t = st.get(name)
                    if t is None:
                        t = strip.tile([128, S], BF16, tag=name, name=name)
                        st[name] = t
                    nc.sync.dma_start_transpose(raw, src[h].bitcast(BF16))
                    nc.sync.dma_start(
                        out=t[64 * j : 64 * (j + 1), :],
                        in_=raw.rearrange("(d h) s -> h d s", h=2)[1],
                    )

            def prep_loads_v(i, st, ones_c):
                # casting swdge DMA: fp32 HBM -> bf16 SBUF, PV weight layout
                for j, h in enumerate((2 * i, 2 * i + 1)):
                    vb = inp.tile(
                        [128, NT, D + 1], BF16, tag="vb", name=f"vb{j}", bufs=4
                    )
                    nc.vector.tensor_copy(vb[:, :, D], ones_c)
                    nc.gpsimd.dma_start(
                        out=vb[:, :, 0:D],
                        in_=v_in[h].rearrange("(t p) d -> p t d", p=128),
                    )
                    st[f"vb{j}"] = vb

            # ---------------- pair-0 prep + consts, interleaved ----------
            st_cur = {}
            raws0 = {}
            for name in ("qT", "kT"):
                st_cur[name] = strip.tile([128, S], BF16, tag=name, name=name)
                for j in range(2):
                    raws0[(name, j)] = strip.tile(
                        [128, S], BF16, tag=f"raw{name}{j}",
                        name=f"raw{name}{j}",
                    )
            # first 512 cols: k strips on the sync queue, q strips on the
            # (startup-idle) scalar queue -> first QK possible at ~4us
            for j in range(2):
                prep_strip_slice(nc.sync, st_cur, raws0, "kT", k_in, j,
                                 slice(0, 512))
                prep_strip_slice(nc.scalar, st_cur, raws0, "qT", q_in, j,
                                 slice(0, 512))

            # consts + v loads while the first XBARs fly
            ones_c = consts.tile([128, NT], FP32)
            nc.vector.memset(ones_c, 1.0)
            prep_loads_v(0, st_cur, ones_c)
            ident = consts.tile([128, 128], FP32)
            make_identity(nc, ident)
            identb = consts.tile([128, 128], BF16)
            nc.vector.tensor_copy(identb, ident)
            tri_f32 = consts.tile([128, 128], FP32)
            make_upper_triangular(nc, tri_f32, val=1.0, diag=True)
            tri = consts.tile([128, 128], BF16)
            nc.vector.tensor_copy(tri, tri_f32)
            warm = consts.tile([1, 8], FP32)
            nc.scalar.activation(
                warm, ones_c[0:1, 0:8], mybir.ActivationFunctionType.Exp
            )

            # PE clock warm-up bridging until the first strips land
            wtp = ps_s.tile([128, 2048], BF16, tag="sP", name="wtp")
            for i in range(WARM_MM):
                off = 512 * (i % 4)
                nc.tensor.transpose(wtp[:, off : off + 128], identb, identb)

            # rest of the pair-0 strips (k on sync, q on scalar)
            for sl in (slice(512, 1024), slice(1024, 2048)):
                for j in range(2):
                    prep_strip_slice(nc.sync, st_cur, raws0, "kT", k_in, j, sl)
                    prep_strip_slice(nc.scalar, st_cur, raws0, "qT", q_in, j,
                                     sl)

            # ---------------- main loop pieces ---------------------------
            sch_state = [0.0, 0.0]  # [total_cols, dve_cols]

            def pick_producer(cols):
                sch_state[0] += cols
                if sch_state[1] < SCH_FRAC * sch_state[0]:
                    sch_state[1] += cols
                    return "dve"
                return "act"

            def emit_qk(st, j, b, m):
                qT, kT = st["qT"], st["kT"]
                o = 64 * j
                cs = (2 * m, 2 * m + 1)
                ts = [c - 4 * b for c in cs]
                j0s = [128 * t if t >= 0 else 0 for t in ts]
                sP = ps_s.tile([128, 2, QB], FP32, tag="sP", name="sP")
                for x in range(2):
                    nc.tensor.matmul(
                        sP[:, x, j0s[x] : QB],
                        kT[o : o + 64, KC * cs[x] : KC * (cs[x] + 1)],
                        qT[o : o + 64, QB * b + j0s[x] : QB * (b + 1)],
                        start=True,
                        stop=True,
                    )
                return sP, j0s, ts

            def emit_exp(sP, j0s, ts):
                pTi = ppool.tile([128, 2, QB], I16, tag="pT", name="pT")
                pT = pTi.bitcast(BF16)
                sPf = sP.rearrange("p a f -> p (a f)")
                pTf = pTi.rearrange("p a f -> p (a f)")
                pTfb = pT.rearrange("p a f -> p (a f)")

                def emit_one(dst_bf, dst_i16, src, cols):
                    # GPSIMD cannot read PSUM, so producers are ACT/DVE only
                    prod = pick_producer(cols)
                    if prod == "act":
                        nc.scalar.activation(
                            dst_bf, src,
                            mybir.ActivationFunctionType.Exp, scale=0.125,
                        )
                    else:
                        nc.vector.tensor_scalar(
                            dst_i16, src, SCH_A, SCH_B,
                            mybir.AluOpType.mult, mybir.AluOpType.add,
                        )

                if j0s[0] == 0:
                    emit_one(
                        pTfb[:, 0 : 2 * QB], pTf[:, 0 : 2 * QB],
                        sPf[:, 0 : 2 * QB], 2 * QB,
                    )
                else:
                    for x in range(2):
                        emit_one(
                            pT[:, x, j0s[x] : QB], pTi[:, x, j0s[x] : QB],
                            sP[:, x, j0s[x] : QB], QB - j0s[x],
                        )
                eng = nc.gpsimd if MASK_POOL else nc.vector
                for x in range(2):
                    if ts[x] >= 0:
                        eng.tensor_mul(
                            pT[:, x, j0s[x] : j0s[x] + 128],
                            pT[:, x, j0s[x] : j0s[x] + 128],
                            tri,
                        )
                return pT

            def emit_pv(vb, oT, pT, j0s, m, npairs):
                cs = (2 * m, 2 * m + 1)
                for x in range(2):
                    nc.tensor.matmul(
                        oT[:, j0s[x] : QB],
                        vb[:, cs[x]],
                        pT[:, x, j0s[x] : QB],
                        start=(m == 0 and x == 0),
                        stop=(m == npairs - 1 and x == 1),
                    )

            def emit_output(h, b, oT):
                oTc = osb.tile([D + 1, QB], BF16, name="oTc")
                nc.vector.tensor_copy(oTc, oT)
                otr = ps_s.tile(
                    [128, 4, D + 1], BF16, tag="sP", name="otr",
                    padded_shape=[128, 4, 512],
                )
                for i in range(4):
                    nc.tensor.transpose(
                        otr[:, i],
                        oTc[:, 128 * i : 128 * (i + 1)],
                        identb[0 : D + 1, 0 : D + 1],
                    )
                rec = res.tile([128, 4], FP32, name="rec")
                nc.vector.reciprocal(rec, otr[:, :, D])
                ores = res.tile([128, 4, D], FP32, name="ores")
                nc.vector.tensor_mul(
                    ores,
                    otr[:, :, 0:D],
                    rec.unsqueeze(2).broadcast_to([128, 4, D]),
                )
                nc.sync.dma_start(
                    out=out_p[h, QB * b : QB * (b + 1), :].rearrange(
                        "(t p) d -> p t d", p=128
                    ),
                    in_=ores,
                )

            # ---------------- schedule -----------------------------------
            NP = HPC // 2
            deferred_prev = []
            pend = []  # (vb, oT, pT, j0s, m, npairs): PV deferred one slot
            for i in range(NP):
                st_nxt = {} if i + 1 < NP else None
                deferred = []
                for b in range(NQB):
                    npairs = 2 * (b + 1)
                    oTs = [
                        ps_o.tile([D + 1, QB], FP32, tag="oT", name=f"oT{j}")
                        for j in range(2)
                    ]
                    gb = i * NQB + b
                    order = [(m, j) for m in range(npairs) for j in range(2)]
                    for m, j in order:
                        sP, j0s, ts = emit_qk(st_cur, j, b, m)
                        pT = emit_exp(sP, j0s, ts)
                        pend.append(
                            (st_cur[f"vb{j}"], oTs[j], pT, j0s, m, npairs, gb)
                        )
                        if len(pend) > PEND_DEPTH:
                            emit_pv(*pend.pop(0)[:6])
                        if m <= 1 and j == 1 and (deferred_prev or deferred):
                            # stagger the two heads' output stages (m=0 and
                            # m=1) so only one otr occupies an sP slot at a
                            # time; drain the previous block's deferred PVs
                            # first or the in-order PE queue deadlocks
                            while pend and pend[0][6] < gb:
                                emit_pv(*pend.pop(0)[:6])
                            todo = deferred_prev + deferred
                            deferred_prev = []
                            deferred = []
                            emit_output(*todo[0])
                            if m == 1 or npairs == 2:
                                for args in todo[1:]:
                                    emit_output(*args)
                            else:
                                deferred = todo[1:]
                    # prep interleave points at block boundaries
                    if st_nxt is not None:
                        if b == 0:
                            prep_strips(i + 1, st_nxt, 0)
                        elif b == 1:
                            prep_loads_v(i + 1, st_nxt, ones_c)
                            prep_strips(i + 1, st_nxt, 1)
                    deferred = [(2 * i + j, b, oTs[j]) for j in range(2)]
                deferred_prev = deferred
                st_cur = st_nxt
            while pend:
                emit_pv(*pend.pop(0)[:6])
            for args in deferred_prev:
                emit_output(*args)

    nc.compile()
    return nc


_NC_CACHE = None
LAST_RESULT = None


def kernel(q: np.ndarray, k: np.ndarray, v: np.ndarray) -> np.ndarray:
    global _NC_CACHE, LAST_RESULT
    if _NC_CACHE is None:
        _NC_CACHE = build_program()
    nc = _NC_CACHE

    def shard(x):
        x = np.ascontiguousarray(np.asarray(x, dtype=np.float32)).reshape(
            B * H, S, D
        )
        return [
            np.ascontiguousarray(x[i * HPC : (i + 1) * HPC])
            for i in range(NCORES)
        ]

    qs, ks, vs = shard(q), shard(k), shard(v)
    ncores = int(os.environ.get("KCORES", str(NCORES)))
    in_maps = [{"q": qs[i], "k": ks[i], "v": vs[i]} for i in range(NCORES)]
    trace = bool(int(os.environ.get("KERNEL_TRACE", "0")))
    result = run_bass_kernel_spmd(
        nc, in_maps[:ncores], core_ids=list(range(ncores)), trace=trace
    )
    LAST_RESULT = result
    outs = [r["out"] for r in result.results]
    if ncores < NCORES:
        outs += [np.zeros((HPC, S, D), np.float32)] * (NCORES - ncores)
    out = np.concatenate(outs, axis=0)
    return out.reshape(B, H, S, D)


# revision 9
# speedup vs baseline: 1.3327x; 1.0040x over previous
"""Causal multi-head attention for Trainium2 (Bass/Tile), 8 NeuronCores. v6.

Problem: q,k,v [B=4, H=16, S=2048, d=64] fp32;
         out = softmax(causal_mask(QK^T/sqrt(d))) @ V.
Sharding: 64 (b,h) head-slices, 8 per core (head parallelism, no comms).

v6 structural changes (from v5 trace analysis):
  - PV is row-split into two K=64 halves on PE tiles T0/T8 (the same
    64x128 row-tiled mode the QK matmuls use), paired across the two
    heads so concurrent tiles always write different PSUM banks:
      slot A: T0 <- h0.khalf0 (bank oT0) || T8 <- h1.khalf1 (bank oT1)
      slot B: T0 <- h1.khalf0 (bank oT1) || T8 <- h0.khalf1 (bank oT0)
    The K-split doubles the PV stream but the 2-tile concurrency wins it
    back, and the QK<->PV tiling-mode switches (a ~128-cycle PE drain
    each, ~2 per (m,j) step, ~50us total) disappear entirely.
  - startup: q strips ride the scalar HWDGE queue (fully issued BEFORE
    any ACT exp), k strips the sync queue, both in 3 consumption-aligned
    512/512/1024-col slices; the first block's exps are forced onto the
    DVE so they don't queue behind the scalar-queue DMAs.  First QK at
    ~4us instead of ~30us (v3) / ~54us (v5).
  - output stores are batched per head ([128,16,64], 2 DMAs/pair instead
    of 8) - fewer DMAs means less false serialization from the DMA
    semaphore recycling protocol.
  - from v5: casting swdge v-load (fp32->bf16 in the DMA), broadcast-AP
    output scale, per-x tri masks on gpsimd, hwdge compacts.
"""

import math
import os

import numpy as np

import concourse.bacc as bacc
import concourse.bass as bass
import concourse.mybir as mybir
from concourse.bass_utils import run_bass_kernel_spmd
from concourse.masks import make_identity, make_upper_triangular
from concourse.tile import TileContext

B, H, S, D = 4, 16, 2048, 64
NCORES = 8
HPC = (B * H) // NCORES  # 8 heads per core
QB = 512                 # q-block (one PSUM bank of fp32)
KC = 128                 # k-chunk
NQB = S // QB            # 4
NT = S // 128            # 16

FP32 = mybir.dt.float32
BF16 = mybir.dt.bfloat16
I16 = mybir.dt.int16

# schraudolph: bf16_bits(exp(x*0.125)) ~= x * SCH_A + SCH_B
SCH_A = 128.0 / math.log(2.0) * 0.125
SCH_B = 16248.5
SCH_FRAC = float(os.environ.get("SCH_FRAC", "0.34"))
WARM_MM = int(os.environ.get("WARM_MM", "8"))
MASK_POOL = bool(int(os.environ.get("MASK_POOL", "1")))
PEND_DEPTH = int(os.environ.get("PEND_DEPTH", "4"))
PV_SPLIT = int(os.environ.get("PV_SPLIT", "1"))
STARTUP = int(os.environ.get("STARTUP", "1"))


def build_program() -> bass.Bass:
    nc = bacc.Bacc(None, target_bir_lowering=False, debug=False)

    q_in = nc.declare_dram_parameter("q", [HPC, S, D], FP32, isOutput=False)
    k_in = nc.declare_dram_parameter("k", [HPC, S, D], FP32, isOutput=False)
    v_in = nc.declare_dram_parameter("v", [HPC, S, D], FP32, isOutput=False)
    out_p = nc.declare_dram_parameter("out", [HPC, S, D], FP32, isOutput=True)

    with TileContext(nc) as tc:
        with (
            tc.tile_pool(name="consts", bufs=1) as consts,
            tc.tile_pool(name="inp", bufs=2) as inp,
            tc.tile_pool(name="strip", bufs=2) as strip,
            tc.tile_pool(name="ppool", bufs=6) as ppool,
            tc.tile_pool(name="osb", bufs=3) as osb,
            tc.tile_pool(name="res", bufs=2) as res,
            tc.tile_pool(name="ps_s", bufs=3, space="PSUM") as ps_s,
            tc.tile_pool(name="ps_o", bufs=2, space="PSUM") as ps_o,
        ):
            # ---------------- prep helpers -------------------------------
            def prep_strip_slice(eng, st, raws, name, src, j, sl):
                """XBAR-transpose one column slice of head j's q/k strip on
                the given HWDGE engine queue, then compact on the same
                queue.

                fp32 [S, 64] bitcast to bf16 [S, 128]: halfword column
                c = 2d+h of row s, h=1 is the truncated-bf16 plane.  The
                XBAR lands column c on partition c; the compact moves the
                odd partitions into the shared strip tile (j=0 ->
                partitions 0:64, j=1 -> 64:128).
                """
                raw = raws[(name, j)]
                eng.dma_start_transpose(raw[:, sl], src[j].bitcast(BF16)[sl, :])
                eng.dma_start(
                    out=st[name][64 * j : 64 * (j + 1), sl],
                    in_=raw.rearrange("(d h) s -> h d s", h=2)[1][:, sl],
                )

            def prep_strips(i, st, j):
                h = 2 * i + j
                for name, src in (("kT", k_in), ("qT", q_in)):
                    raw = strip.tile([128, S], BF16, tag=f"raw{name}{j}",
                                     name=f"raw{name}{j}")
                    t = st.get(name)
                    if t is None:
                        t = strip.tile([128, S], BF16, tag=name, name=name)
                        st[name] = t
                    nc.sync.dma_start_transpose(raw, src[h].bitcast(BF16))
                    nc.sync.dma_start(
                        out=t[64 * j : 64 * (j + 1), :],
                        in_=raw.rearrange("(d h) s -> h d s", h=2)[1],
                    )

            def prep_loads_v(i, st, ones_c):
                # casting swdge DMA: fp32 HBM -> bf16 SBUF, PV weight layout
                for j, h in enumerate((2 * i, 2 * i + 1)):
                    vb = inp.tile(
                        [128, NT, D + 1], BF16, tag="vb", name=f"vb{j}", bufs=4
                    )
                    nc.vector.tensor_copy(vb[:, :, D], ones_c)
                    nc.gpsimd.dma_start(
                        out=vb[:, :, 0:D],
                        in_=v_in[h].rearrange("(t p) d -> p t d", p=128),
                    )
                    st[f"vb{j}"] = vb

            # ---------------- pair-0 prep + consts, interleaved ----------
            st_cur = {}
            raws0 = {}
            for name in ("qT", "kT"):
                st_cur[name] = strip.tile([128, S], BF16, tag=name, name=name)
                for j in range(2):
                    raws0[(name, j)] = strip.tile(
                        [128, S], BF16, tag=f"raw{name}{j}",
                        name=f"raw{name}{j}",
                    )
            # consumption-aligned slices; k on sync, q on the scalar HWDGE
            # queue (which must stay DMA-only until these finish - the
            # first block's exps are forced onto the DVE below)
            if STARTUP:
                for sl in (slice(0, 512), slice(512, 1024),
                           slice(1024, 2048)):
                    for j in range(2):
                        prep_strip_slice(nc.sync, st_cur, raws0, "kT", k_in,
                                         j, sl)
                        prep_strip_slice(nc.scalar, st_cur, raws0, "qT", q_in,
                                         j, sl)
            else:
                # v3-style: halves, everything on sync
                for sl in (slice(0, 1024), slice(1024, 2048)):
                    for j in range(2):
                        prep_strip_slice(nc.sync, st_cur, raws0, "kT", k_in,
                                         j, sl)
                        prep_strip_slice(nc.sync, st_cur, raws0, "qT", q_in,
                                         j, sl)

            # consts + v loads while the XBARs fly
            ones_c = consts.tile([128, NT], FP32)
            nc.vector.memset(ones_c, 1.0)
            ident = consts.tile([128, 128], FP32)
            make_identity(nc, ident)
            identb = consts.tile([128, 128], BF16)
            nc.vector.tensor_copy(identb, ident)
            tri_f32 = consts.tile([128, 128], FP32)
            make_upper_triangular(nc, tri_f32, val=1.0, diag=True)
            tri = consts.tile([128, 128], BF16)
            nc.vector.tensor_copy(tri, tri_f32)
            prep_loads_v(0, st_cur, ones_c)

            # PE clock warm-up bridging until the first strips land
            wtp = ps_s.tile([128, 2048], BF16, tag="sP", name="wtp")
            for i in range(WARM_MM):
                off = 512 * (i % 4)
                nc.tensor.transpose(wtp[:, off : off + 128], identb, identb)

            # ---------------- main loop pieces ---------------------------
            sch_state = [0.0, 0.0]  # [total_cols, dve_cols]

            def pick_producer(cols, force=None):
                sch_state[0] += cols
                if force == "dve" or (
                    force is None and sch_state[1] < SCH_FRAC * sch_state[0]
                ):
                    sch_state[1] += cols
                    return "dve"
                return "act"

            def emit_qk(st, j, b, m):
                qT, kT = st["qT"], st["kT"]
                o = 64 * j
                cs = (2 * m, 2 * m + 1)
                ts = [c - 4 * b for c in cs]
                j0s = [128 * t if t >= 0 else 0 for t in ts]
                sP = ps_s.tile([128, 2, QB], FP32, tag="sP", name="sP")
                for x in range(2):
                    nc.tensor.matmul(
                        sP[:, x, j0s[x] : QB],
                        kT[o : o + 64, KC * cs[x] : KC * (cs[x] + 1)],
                        qT[o : o + 64, QB * b + j0s[x] : QB * (b + 1)],
                        start=True,
                        stop=True,
                    )
                return sP, j0s, ts

            def emit_exp(sP, j0s, ts, force=None):
                pTi = ppool.tile([128, 2, QB], I16, tag="pT", name="pT")
                pT = pTi.bitcast(BF16)
                sPf = sP.rearrange("p a f -> p (a f)")
                pTf = pTi.rearrange("p a f -> p (a f)")
                pTfb = pT.rearrange("p a f -> p (a f)")

                def emit_one(dst_bf, dst_i16, src, cols):
                    # GPSIMD cannot read PSUM, so producers are ACT/DVE only
                    prod = pick_producer(cols, force)
                    if prod == "act":
                        nc.scalar.activation(
                            dst_bf, src,
                            mybir.ActivationFunctionType.Exp, scale=0.125,
                        )
                    else:
                        nc.vector.tensor_scalar(
                            dst_i16, src, SCH_A, SCH_B,
                            mybir.AluOpType.mult, mybir.AluOpType.add,
                        )

                if j0s[0] == 0:
                    emit_one(
                        pTfb[:, 0 : 2 * QB], pTf[:, 0 : 2 * QB],
                        sPf[:, 0 : 2 * QB], 2 * QB,
                    )
                else:
                    for x in range(2):
                        emit_one(
                            pT[:, x, j0s[x] : QB], pTi[:, x, j0s[x] : QB],
                            sP[:, x, j0s[x] : QB], QB - j0s[x],
                        )
                eng = nc.gpsimd if MASK_POOL else nc.vector
                for x in range(2):
                    if ts[x] >= 0:
                        eng.tensor_mul(
                            pT[:, x, j0s[x] : j0s[x] + 128],
                            pT[:, x, j0s[x] : j0s[x] + 128],
                            tri,
                        )
                return pT

            def emit_pv2(e0, e1):
                """Row-split PV for a head pair: K=64 halves on tiles T0/T8,
                cross-paired so concurrent tiles write different banks.

                e = (vb, oT, pT, j0s, m, npairs) for j=0 (e0) and j=1 (e1).
                """
                for x in range(2):
                    for ph in range(2):
                        # ph 0: T0 <- e0.half0, T8 <- e1.half1
                        # ph 1: T0 <- e1.half0, T8 <- e0.half1
                        # ph 0 emits each head's FIRST half for this x,
                        # ph 1 its second, so start/stop key off ph.
                        for half, e in ((0, (e0, e1)[ph]), (1, (e1, e0)[ph])):
                            vb, oT, pT, j0s, m, npairs = e
                            c = 2 * m + x
                            o = 64 * half
                            first = m == 0 and x == 0 and ph == 0
                            last = m == npairs - 1 and x == 1 and ph == 1
                            nc.tensor.matmul(
                                oT[:, j0s[x] : QB],
                                vb[o : o + 64, c],
                                pT[o : o + 64, x, j0s[x] : QB],
                                start=first,
                                stop=last,
                            )

            def emit_pv_serial(vb, oT, pT, j0s, m, npairs):
                cs = (2 * m, 2 * m + 1)
                for x in range(2):
                    nc.tensor.matmul(
                        oT[:, j0s[x] : QB],
                        vb[:, cs[x]],
                        pT[:, x, j0s[x] : QB],
                        start=(m == 0 and x == 0),
                        stop=(m == npairs - 1 and x == 1),
                    )

            def pop_pv2(pend):
                e0 = pend.pop(0)
                e1 = pend.pop(0)
                if PV_SPLIT:
                    emit_pv2(e0[:6], e1[:6])
                else:
                    emit_pv_serial(*e0[:6])
                    emit_pv_serial(*e1[:6])

            def emit_output(h, b, oT, ores_h):
                oTc = osb.tile([D + 1, QB], BF16, name="oTc")
                nc.vector.tensor_copy(oTc, oT)
                otr = ps_s.tile(
                    [128, 4, D + 1], BF16, tag="sP", name="otr",
                    padded_shape=[128, 4, 512],
                )
                for i in range(4):
                    nc.tensor.transpose(
                        otr[:, i],
                        oTc[:, 128 * i : 128 * (i + 1)],
                        identb[0 : D + 1, 0 : D + 1],
                    )
                rec = res.tile([128, 4], FP32, name="rec", bufs=4)
                nc.vector.reciprocal(rec, otr[:, :, D])
                nc.vector.tensor_mul(
                    ores_h[:, 4 * b : 4 * (b + 1)],
                    otr[:, :, 0:D],
                    rec.unsqueeze(2).broadcast_to([128, 4, D]),
                )
                if b == NQB - 1:
                    # whole head accumulated: one batched store
                    nc.sync.dma_start(
                        out=out_p[h].rearrange("(t p) d -> p t d", p=128),
                        in_=ores_h,
                    )

            # ---------------- schedule -----------------------------------
            NP = HPC // 2
            deferred_prev = []
            pend = []  # (vb, oT, pT, j0s, m, npairs, gb): PV deferred
            ores_hs = {}
            for i in range(NP):
                st_nxt = {} if i + 1 < NP else None
                deferred = []
                for j in range(2):
                    ores_hs[2 * i + j] = res.tile(
                        [128, NT, D], FP32, tag=f"ores{j}", name=f"ores{j}"
                    )
                for b in range(NQB):
                    npairs = 2 * (b + 1)
                    oTs = [
                        ps_o.tile([D + 1, QB], FP32, tag="oT", name=f"oT{j}")
                        for j in range(2)
                    ]
                    gb = i * NQB + b
                    force = "dve" if gb == 0 else None
                    order = [(m, j) for m in range(npairs) for j in range(2)]
                    for m, j in order:
                        sP, j0s, ts = emit_qk(st_cur, j, b, m)
                        pT = emit_exp(sP, j0s, ts, force)
                        pend.append(
                            (st_cur[f"vb{j}"], oTs[j], pT, j0s, m, npairs, gb)
                        )
                        if len(pend) > PEND_DEPTH and j == 1:
                            pop_pv2(pend)
                        if m <= 1 and j == 1 and (deferred_prev or deferred):
                            # stagger the two heads' output stages (m=0 and
                            # m=1) so only one otr occupies an sP slot at a
                            # time; drain the previous block's deferred PVs
                            # first or the in-order PE queue deadlocks
                            while pend and pend[0][6] < gb:
                                pop_pv2(pend)
                            todo = deferred_prev + deferred
                            deferred_prev = []
                            deferred = []
                            emit_output(*todo[0])
                            if m == 1 or npairs == 2:
                                for args in todo[1:]:
                                    emit_output(*args)
                            else:
                                deferred = todo[1:]
                    # prep interleave points at block boundaries
                    if st_nxt is not None:
                        if b == 0:
                            prep_strips(i + 1, st_nxt, 0)
                        elif b == 1:
                            prep_loads_v(i + 1, st_nxt, ones_c)
                            prep_strips(i + 1, st_nxt, 1)
                    deferred = [
                        (2 * i + j, b, oTs[j], ores_hs[2 * i + j])
                        for j in range(2)
                    ]
                deferred_prev = deferred
                st_cur = st_nxt
            while pend:
                pop_pv2(pend)
            for args in deferred_prev:
                emit_output(*args)

    nc.compile()
    return nc


_NC_CACHE = None
LAST_RESULT = None


def kernel(q: np.ndarray, k: np.ndarray, v: np.ndarray) -> np.ndarray:
    global _NC_CACHE, LAST_RESULT
    if _NC_CACHE is None:
        _NC_CACHE = build_program()
    nc = _NC_CACHE

    def shard(x):
        x = np.ascontiguousarray(np.asarray(x, dtype=np.float32)).reshape(
            B * H, S, D
        )
        return [
            np.ascontiguousarray(x[i * HPC : (i + 1) * HPC])
            for i in range(NCORES)
        ]

    qs, ks, vs = shard(q), shard(k), shard(v)
    ncores = int(os.environ.get("KCORES", str(NCORES)))
    in_maps = [{"q": qs[i], "k": ks[i], "v": vs[i]} for i in range(NCORES)]
    trace = bool(int(os.environ.get("KERNEL_TRACE", "0")))
    result = run_bass_kernel_spmd(
        nc, in_maps[:ncores], core_ids=list(range(ncores)), trace=trace
    )
    LAST_RESULT = result
    outs = [r["out"] for r in result.results]
    if ncores < NCORES:
        outs += [np.zeros((HPC, S, D), np.float32)] * (NCORES - ncores)
    out = np.concatenate(outs, axis=0)
    return out.reshape(B, H, S, D)


# revision 12
# speedup vs baseline: 1.3625x; 1.0224x over previous
"""Causal multi-head attention for Trainium2 (Bass/Tile), 8 NeuronCores. v6.

Problem: q,k,v [B=4, H=16, S=2048, d=64] fp32;
         out = softmax(causal_mask(QK^T/sqrt(d))) @ V.
Sharding: 64 (b,h) head-slices, 8 per core (head parallelism, no comms).

v6 structural changes (from v5 trace analysis):
  - PV is row-split into two K=64 halves on PE tiles T0/T8 (the same
    64x128 row-tiled mode the QK matmuls use), paired across the two
    heads so concurrent tiles always write different PSUM banks:
      slot A: T0 <- h0.khalf0 (bank oT0) || T8 <- h1.khalf1 (bank oT1)
      slot B: T0 <- h1.khalf0 (bank oT1) || T8 <- h0.khalf1 (bank oT0)
    The K-split doubles the PV stream but the 2-tile concurrency wins it
    back, and the QK<->PV tiling-mode switches (a ~128-cycle PE drain
    each, ~2 per (m,j) step, ~50us total) disappear entirely.
  - startup: q strips ride the scalar HWDGE queue (fully issued BEFORE
    any ACT exp), k strips the sync queue, both in 3 consumption-aligned
    512/512/1024-col slices; the first block's exps are forced onto the
    DVE so they don't queue behind the scalar-queue DMAs.  First QK at
    ~4us instead of ~30us (v3) / ~54us (v5).
  - output stores are batched per head ([128,16,64], 2 DMAs/pair instead
    of 8) - fewer DMAs means less false serialization from the DMA
    semaphore recycling protocol.
  - from v5: casting swdge v-load (fp32->bf16 in the DMA), broadcast-AP
    output scale, per-x tri masks on gpsimd, hwdge compacts.
"""

import math
import os

import numpy as np

import concourse.bacc as bacc
import concourse.bass as bass
import concourse.mybir as mybir
from concourse.bass_utils import run_bass_kernel_spmd
from concourse.masks import make_identity, make_upper_triangular
from concourse.tile import TileContext

B, H, S, D = 4, 16, 2048, 64
NCORES = 8
HPC = (B * H) // NCORES  # 8 heads per core
QB = 512                 # q-block (one PSUM bank of fp32)
KC = 128                 # k-chunk
NQB = S // QB            # 4
NT = S // 128            # 16

FP32 = mybir.dt.float32
BF16 = mybir.dt.bfloat16
I16 = mybir.dt.int16

# schraudolph: bf16_bits(exp(x*0.125)) ~= x * SCH_A + SCH_B
SCH_A = 128.0 / math.log(2.0) * 0.125
SCH_B = 16248.5
SCH_FRAC = float(os.environ.get("SCH_FRAC", "0.34"))
WARM_MM = int(os.environ.get("WARM_MM", "8"))
MASK_POOL = bool(int(os.environ.get("MASK_POOL", "1")))
PEND_DEPTH = int(os.environ.get("PEND_DEPTH", "4"))
PV_SPLIT = int(os.environ.get("PV_SPLIT", "1"))
STARTUP = int(os.environ.get("STARTUP", "1"))


def build_program() -> bass.Bass:
    nc = bacc.Bacc(None, target_bir_lowering=False, debug=False)

    q_in = nc.declare_dram_parameter("q", [HPC, S, D], FP32, isOutput=False)
    k_in = nc.declare_dram_parameter("k", [HPC, S, D], FP32, isOutput=False)
    v_in = nc.declare_dram_parameter("v", [HPC, S, D], FP32, isOutput=False)
    out_p = nc.declare_dram_parameter("out", [HPC, S, D], FP32, isOutput=True)

    with TileContext(nc) as tc:
        with (
            tc.tile_pool(name="consts", bufs=1) as consts,
            tc.tile_pool(name="inp", bufs=2) as inp,
            tc.tile_pool(name="strip", bufs=2) as strip,
            tc.tile_pool(name="ppool", bufs=6) as ppool,
            tc.tile_pool(name="osb", bufs=3) as osb,
            tc.tile_pool(name="res", bufs=2) as res,
            tc.tile_pool(name="ps_s", bufs=3, space="PSUM") as ps_s,
            tc.tile_pool(name="ps_o", bufs=2, space="PSUM") as ps_o,
        ):
            # ---------------- prep helpers -------------------------------
            def prep_strip_slice(eng, st, raws, name, src, j, sl):
                """XBAR-transpose one column slice of head j's q/k strip on
                the given HWDGE engine queue, then compact on the same
                queue.

                fp32 [S, 64] bitcast to bf16 [S, 128]: halfword column
                c = 2d+h of row s, h=1 is the truncated-bf16 plane.  The
                XBAR lands column c on partition c; the compact moves the
                odd partitions into the shared strip tile (j=0 ->
                partitions 0:64, j=1 -> 64:128).
                """
                raw = raws[(name, j)]
                eng.dma_start_transpose(raw[:, sl], src[j].bitcast(BF16)[sl, :])
                eng.dma_start(
                    out=st[name][64 * j : 64 * (j + 1), sl],
                    in_=raw.rearrange("(d h) s -> h d s", h=2)[1][:, sl],
                )

            def prep_strips(i, st, j):
                h = 2 * i + j
                for name, src in (("kT", k_in), ("qT", q_in)):
                    raw = strip.tile([128, S], BF16, tag=f"raw{name}{j}",
                                     name=f"raw{name}{j}")
                    t = st.get(name)
                    if t is None:
                        t = strip.tile([128, S], BF16, tag=name, name=name)
                        st[name] = t
                    nc.sync.dma_start_transpose(raw, src[h].bitcast(BF16))
                    nc.sync.dma_start(
                        out=t[64 * j : 64 * (j + 1), :],
                        in_=raw.rearrange("(d h) s -> h d s", h=2)[1],
                    )

            def prep_loads_v(i, st, ones_c):
                # casting swdge DMA: fp32 HBM -> bf16 SBUF, PV weight layout
                for j, h in enumerate((2 * i, 2 * i + 1)):
                    vb = inp.tile(
                        [128, NT, D + 1], BF16, tag="vb", name=f"vb{j}", bufs=4
                    )
                    nc.vector.tensor_copy(vb[:, :, D], ones_c)
                    nc.gpsimd.dma_start(
                        out=vb[:, :, 0:D],
                        in_=v_in[h].rearrange("(t p) d -> p t d", p=128),
                    )
                    st[f"vb{j}"] = vb

            # ---------------- pair-0 prep + consts, interleaved ----------
            st_cur = {}
            raws0 = {}
            for name in ("qT", "kT"):
                st_cur[name] = strip.tile([128, S], BF16, tag=name, name=name)
                for j in range(2):
                    raws0[(name, j)] = strip.tile(
                        [128, S], BF16, tag=f"raw{name}{j}",
                        name=f"raw{name}{j}",
                    )
            # consumption-aligned slices; k on sync, q on the scalar HWDGE
            # queue (which must stay DMA-only until these finish - the
            # first block's exps are forced onto the DVE below)
            if STARTUP == 2:
                # fine slices, all on sync
                for sl in (slice(0, 512), slice(512, 1024),
                           slice(1024, 2048)):
                    for j in range(2):
                        prep_strip_slice(nc.sync, st_cur, raws0, "kT", k_in,
                                         j, sl)
                        prep_strip_slice(nc.sync, st_cur, raws0, "qT", q_in,
                                         j, sl)
            elif STARTUP == 1:
                for sl in (slice(0, 512), slice(512, 1024),
                           slice(1024, 2048)):
                    for j in range(2):
                        prep_strip_slice(nc.sync, st_cur, raws0, "kT", k_in,
                                         j, sl)
                        prep_strip_slice(nc.scalar, st_cur, raws0, "qT", q_in,
                                         j, sl)
            else:
                # v3-style: halves, everything on sync
                for sl in (slice(0, 1024), slice(1024, 2048)):
                    for j in range(2):
                        prep_strip_slice(nc.sync, st_cur, raws0, "kT", k_in,
                                         j, sl)
                        prep_strip_slice(nc.sync, st_cur, raws0, "qT", q_in,
                                         j, sl)

            # consts + v loads while the XBARs fly
            ones_c = consts.tile([128, NT], FP32)
            nc.vector.memset(ones_c, 1.0)
            ident = consts.tile([128, 128], FP32)
            make_identity(nc, ident)
            identb = consts.tile([128, 128], BF16)
            nc.vector.tensor_copy(identb, ident)
            tri_f32 = consts.tile([128, 128], FP32)
            make_upper_triangular(nc, tri_f32, val=1.0, diag=True)
            tri = consts.tile([128, 128], BF16)
            nc.vector.tensor_copy(tri, tri_f32)
            prep_loads_v(0, st_cur, ones_c)

            # PE clock warm-up bridging until the first strips land
            wtp = ps_s.tile([128, 2048], BF16, tag="sP", name="wtp")
            for i in range(WARM_MM):
                off = 512 * (i % 4)
                nc.tensor.transpose(wtp[:, off : off + 128], identb, identb)

            # ---------------- main loop pieces ---------------------------
            sch_state = [0.0, 0.0]  # [total_cols, dve_cols]

            def pick_producer(cols, force=None):
                sch_state[0] += cols
                if force == "dve" or (
                    force is None and sch_state[1] < SCH_FRAC * sch_state[0]
                ):
                    sch_state[1] += cols
                    return "dve"
                return "act"

            def emit_qk_pair(st, b, m):
                """QK for both heads of chunk-pair m, x-major with j
                alternating so consecutive mms land on tiles T0/T8 and run
                concurrently (2 slots per m instead of 3)."""
                qT, kT = st["qT"], st["kT"]
                cs = (2 * m, 2 * m + 1)
                ts = [c - 4 * b for c in cs]
                j0s = [128 * t if t >= 0 else 0 for t in ts]
                sPs = [
                    ps_s.tile([128, 2, QB], FP32, tag="sP", name=f"sP{j}")
                    for j in range(2)
                ]
                for x in range(2):
                    for j in range(2):
                        o = 64 * j
                        nc.tensor.matmul(
                            sPs[j][:, x, j0s[x] : QB],
                            kT[o : o + 64, KC * cs[x] : KC * (cs[x] + 1)],
                            qT[o : o + 64, QB * b + j0s[x] : QB * (b + 1)],
                            start=True,
                            stop=True,
                        )
                return sPs, j0s, ts

            def emit_exp(sP, j0s, ts, force=None):
                pTi = ppool.tile([128, 2, QB], I16, tag="pT", name="pT")
                pT = pTi.bitcast(BF16)
                sPf = sP.rearrange("p a f -> p (a f)")
                pTf = pTi.rearrange("p a f -> p (a f)")
                pTfb = pT.rearrange("p a f -> p (a f)")

                def emit_one(dst_bf, dst_i16, src, cols):
                    # GPSIMD cannot read PSUM, so producers are ACT/DVE only
                    prod = pick_producer(cols, force)
                    if prod == "act":
                        nc.scalar.activation(
                            dst_bf, src,
                            mybir.ActivationFunctionType.Exp, scale=0.125,
                        )
                    else:
                        nc.vector.tensor_scalar(
                            dst_i16, src, SCH_A, SCH_B,
                            mybir.AluOpType.mult, mybir.AluOpType.add,
                        )

                if j0s[0] == 0:
                    emit_one(
                        pTfb[:, 0 : 2 * QB], pTf[:, 0 : 2 * QB],
                        sPf[:, 0 : 2 * QB], 2 * QB,
                    )
                else:
                    for x in range(2):
                        emit_one(
                            pT[:, x, j0s[x] : QB], pTi[:, x, j0s[x] : QB],
                            sP[:, x, j0s[x] : QB], QB - j0s[x],
                        )
                eng = nc.gpsimd if MASK_POOL else nc.vector
                for x in range(2):
                    if ts[x] >= 0:
                        eng.tensor_mul(
                            pT[:, x, j0s[x] : j0s[x] + 128],
                            pT[:, x, j0s[x] : j0s[x] + 128],
                            tri,
                        )
                return pT

            def emit_pv2(e0, e1):
                """Row-split PV for a head pair: K=64 halves on tiles T0/T8,
                cross-paired so concurrent tiles write different banks.

                e = (vb, oT, pT, j0s, m, npairs) for j=0 (e0) and j=1 (e1).
                """
                for x in range(2):
                    for ph in range(2):
                        # ph 0: T0 <- e0.half0, T8 <- e1.half1
                        # ph 1: T0 <- e1.half0, T8 <- e0.half1
                        # ph 0 emits each head's FIRST half for this x,
                        # ph 1 its second, so start/stop key off ph.
                        for half, e in ((0, (e0, e1)[ph]), (1, (e1, e0)[ph])):
                            vb, oT, pT, j0s, m, npairs = e
                            c = 2 * m + x
                            o = 64 * half
                            first = m == 0 and x == 0 and ph == 0
                            last = m == npairs - 1 and x == 1 and ph == 1
                            nc.tensor.matmul(
                                oT[:, j0s[x] : QB],
                                vb[o : o + 64, c],
                                pT[o : o + 64, x, j0s[x] : QB],
                                start=first,
                                stop=last,
                            )

            def emit_pv_serial(vb, oT, pT, j0s, m, npairs):
                cs = (2 * m, 2 * m + 1)
                for x in range(2):
                    nc.tensor.matmul(
                        oT[:, j0s[x] : QB],
                        vb[:, cs[x]],
                        pT[:, x, j0s[x] : QB],
                        start=(m == 0 and x == 0),
                        stop=(m == npairs - 1 and x == 1),
                    )

            def pop_pv2(pend):
                e0 = pend.pop(0)
                e1 = pend.pop(0)
                if PV_SPLIT:
                    emit_pv2(e0[:6], e1[:6])
                else:
                    emit_pv_serial(*e0[:6])
                    emit_pv_serial(*e1[:6])

            def emit_output(h, b, oT, ores_h):
                oTc = osb.tile([D + 1, QB], BF16, name="oTc")
                nc.vector.tensor_copy(oTc, oT)
                otr = ps_s.tile(
                    [128, 4, D + 1], BF16, tag="sP", name="otr",
                    padded_shape=[128, 4, 512],
                )
                for i in range(4):
                    nc.tensor.transpose(
                        otr[:, i],
                        oTc[:, 128 * i : 128 * (i + 1)],
                        identb[0 : D + 1, 0 : D + 1],
                    )
                rec = res.tile([128, 4], FP32, name="rec", bufs=4)
                nc.vector.reciprocal(rec, otr[:, :, D])
                nc.vector.tensor_mul(
                    ores_h[:, 4 * b : 4 * (b + 1)],
                    otr[:, :, 0:D],
                    rec.unsqueeze(2).broadcast_to([128, 4, D]),
                )
                if b == NQB - 1:
                    # whole head accumulated: one batched store
                    nc.sync.dma_start(
                        out=out_p[h].rearrange("(t p) d -> p t d", p=128),
                        in_=ores_h,
                    )

            # ---------------- schedule -----------------------------------
            NP = HPC // 2
            deferred_prev = []
            pend = []  # (vb, oT, pT, j0s, m, npairs, gb): PV deferred
            ores_hs = {}
            for i in range(NP):
                st_nxt = {} if i + 1 < NP else None
                deferred = []
                for j in range(2):
                    ores_hs[2 * i + j] = res.tile(
                        [128, NT, D], FP32, tag=f"ores{j}", name=f"ores{j}"
                    )
                for b in range(NQB):
                    npairs = 2 * (b + 1)
                    oTs = [
                        ps_o.tile([D + 1, QB], FP32, tag="oT", name=f"oT{j}")
                        for j in range(2)
                    ]
                    gb = i * NQB + b
                    force = "dve" if gb == 0 else None
                    for m in range(npairs):
                        sPs, j0s, ts = emit_qk_pair(st_cur, b, m)
                        for j in range(2):
                            pT = emit_exp(sPs[j], j0s, ts, force)
                            pend.append(
                                (st_cur[f"vb{j}"], oTs[j], pT, j0s, m,
                                 npairs, gb)
                            )
                        j = 1
                        if len(pend) > PEND_DEPTH:
                            pop_pv2(pend)
                        if m <= 1 and (deferred_prev or deferred):
                            # stagger the two heads' output stages (m=0 and
                            # m=1) so only one otr occupies an sP slot at a
                            # time; drain the previous block's deferred PVs
                            # first or the in-order PE queue deadlocks
                            while pend and pend[0][6] < gb:
                                pop_pv2(pend)
                            todo = deferred_prev + deferred
                            deferred_prev = []
                            deferred = []
                            emit_output(*todo[0])
                            if m == 1 or npairs == 2:
                                for args in todo[1:]:
                                    emit_output(*args)
                            else:
                                deferred = todo[1:]
                    # prep interleave points at block boundaries
                    if st_nxt is not None:
                        if b == 0:
                            prep_strips(i + 1, st_nxt, 0)
                        elif b == 1:
                            prep_loads_v(i + 1, st_nxt, ones_c)
                            prep_strips(i + 1, st_nxt, 1)
                    deferred = [
                        (2 * i + j, b, oTs[j], ores_hs[2 * i + j])
                        for j in range(2)
                    ]
                deferred_prev = deferred
                st_cur = st_nxt
            while pend:
                pop_pv2(pend)
            for args in deferred_prev:
                emit_output(*args)

    nc.compile()
    return nc


_NC_CACHE = None
LAST_RESULT = None


def kernel(q: np.ndarray, k: np.ndarray, v: np.ndarray) -> np.ndarray:
    global _NC_CACHE, LAST_RESULT
    if _NC_CACHE is None:
        _NC_CACHE = build_program()
    nc = _NC_CACHE

    def shard(x):
        x = np.ascontiguousarray(np.asarray(x, dtype=np.float32)).reshape(
            B * H, S, D
        )
        return [
            np.ascontiguousarray(x[i * HPC : (i + 1) * HPC])
            for i in range(NCORES)
        ]

    qs, ks, vs = shard(q), shard(k), shard(v)
    ncores = int(os.environ.get("KCORES", str(NCORES)))
    in_maps = [{"q": qs[i], "k": ks[i], "v": vs[i]} for i in range(NCORES)]
    trace = bool(int(os.environ.get("KERNEL_TRACE", "0")))
    result = run_bass_kernel_spmd(
        nc, in_maps[:ncores], core_ids=list(range(ncores)), trace=trace
    )
    LAST_RESULT = result
    outs = [r["out"] for r in result.results]
    if ncores < NCORES:
        outs += [np.zeros((HPC, S, D), np.float32)] * (NCORES - ncores)
    out = np.concatenate(outs, axis=0)
    return out.reshape(B, H, S, D)
